# revision 16
# baseline (speedup 1.0000x reference)
"""Bahdanau-style attention kernel for Trainium2, data-parallel over 8 NeuronCores.

Reference computation (per batch b):
    hidden  = lstm_hidden_state[-1]                      # [B, H]
    h_proj  = hidden @ attn_w[:, :H].T + attn_b          # [B, H]
    o_proj  = lstm_outputs @ attn_w[:, H:].T             # [B, S, H]
    energy  = tanh(o_proj + h_proj[:, None, :])          # [B, S, H]
    scores  = energy @ v_w                               # [B, S]
    alpha   = softmax(scores, axis=1)                    # [B, S]
    context = einsum('bs,bsh->bh', alpha, lstm_outputs)  # [B, H]
    returns (context, alpha)

Strategy:
  - Shard batch B=64 across 8 cores (8 batches each); weights replicated.
  - Host pre-transposes lstm_outputs to [B, H, S] so the device streams
    contiguous [128, S] tiles once (single pass over the 33.5MB/core shard).
  - o_projT[k, s] via f32r matmuls (W2T chunks stationary).
  - tanh(+h_proj bias) on ScalarE, PSUM -> SBUF, 2048-wide ops.
  - scores via a replicated-v stationary operand so the score row comes out
    broadcast across all 128 partitions (needed by the context reduce).
  - p = exp(scores) on ScalarE (scores are bounded by ||v||_1 <= 16, so no
    max-subtraction is needed; softmax normalization happens on host).
  - context = sum_s xT[h, s] * p[s] as a fused DVE tensor_tensor_reduce.
"""

import os
import sys
import numpy as np

sys.path.insert(0, "/opt/trn_rl_repo")

B, S, H = 64, 4096, 256
NCORES = 8
BC = B // NCORES          # batches per core
HC = H // 128             # 128-partition h/k chunks per core
JG = 2048                 # free-dim width of ACT/DVE ops
NJG = S // JG
J4 = 512                  # matmul moving-operand width (fp32 max)
NJ4 = JG // J4

_BUILT = None
LAST_EXEC_TIME_NS = None


def _round_fp32r(x):
    """Round fp32 to the PE's fp32r (tf32-like) format: round-to-nearest-even
    at mantissa bit 12, low 12 bits zeroed."""
    u = np.ascontiguousarray(x, dtype=np.float32).view(np.uint32)
    keep = u & np.uint32(0xFFFFF000)
    rem = u & np.uint32(0x00000FFF)
    half = np.uint32(0x800)
    lsb = (u >> np.uint32(12)) & np.uint32(1)
    round_up = (rem > half) | ((rem == half) & (lsb == 1))
    out = keep + (round_up.astype(np.uint32) << np.uint32(12))
    return out.view(np.float32)


def _build():
    import concourse.tile as tile
    from concourse import bacc, mybir

    f32 = mybir.dt.float32
    f32r = mybir.dt.float32r
    fp16 = mybir.dt.float16
    Tanh = mybir.ActivationFunctionType.Tanh
    Exp = mybir.ActivationFunctionType.Exp
    mult = mybir.AluOpType.mult
    add = mybir.AluOpType.add

    nc = bacc.Bacc(
        "TRN2", target_bir_lowering=False, debug=False, num_devices=NCORES
    )

    xt = nc.dram_tensor("xt", [BC, HC, 128, S], f32r, kind="ExternalInput").ap()
    w2t = nc.dram_tensor("w2t", [H, H], f32r, kind="ExternalInput").ap()
    vrep = nc.dram_tensor("vrep", [H, 128], fp16, kind="ExternalInput").ap()
    hproj = nc.dram_tensor("hproj", [H, BC], f32, kind="ExternalInput").ap()
    alpha_u = nc.dram_tensor("alpha_u", [BC, S], f32, kind="ExternalOutput").ap()
    ctxu = nc.dram_tensor("ctxu", [128, HC * BC], f32, kind="ExternalOutput").ap()

    with tile.TileContext(nc) as tc:
        with (
            tc.tile_pool(name="singles", bufs=1) as singles,
            tc.tile_pool(name="xtp", bufs=12) as xtp,
            tc.tile_pool(name="enp", bufs=6) as enp,
            tc.tile_pool(name="pbp", bufs=4) as pbp,
            tc.tile_pool(name="scrp", bufs=2) as scrp,
            tc.tile_pool(name="accp", bufs=8) as accp,
            tc.tile_pool(name="psp", bufs=2, space="PSUM") as psp,
        ):
            # weights on the scalar HWDGE queue (issues in parallel with sync)
            w2t_sb = []
            vrep_sb = []
            hproj_sb = []
            for hc in range(HC):
                t = singles.tile([128, H], f32r, tag=f"w2t{hc}")
                nc.scalar.dma_start(t[:], w2t[hc * 128:(hc + 1) * 128, :])
                w2t_sb.append(t)
            for hc in range(HC):
                t = singles.tile([128, 128], fp16, tag=f"vrep{hc}")
                nc.scalar.dma_start(t[:], vrep[hc * 128:(hc + 1) * 128, :])
                vrep_sb.append(t)
                t = singles.tile([128, BC], f32, tag=f"hproj{hc}")
                nc.scalar.dma_start(t[:], hproj[hc * 128:(hc + 1) * 128, :])
                hproj_sb.append(t)
            ctx_sb = singles.tile([128, HC * BC], f32, tag="ctx")

            from concourse.dve_ops import TENSOR_TENSOR_REDUCE

            xt_first = {}  # (hc, j4) -> [128, J4] tile, for (b=0, jg=0)
            for j4 in range(NJ4):
                for hc in range(HC):
                    t = xtp.tile([128, J4], f32r, tag="xtf")
                    nc.sync.dma_start(
                        t[:], xt[0, hc][:, j4 * J4:(j4 + 1) * J4])
                    xt_first[(hc, j4)] = t

            for b in range(BC):
                xt_c = {}
                for jg in range(NJG):
                    if b == 0 and jg == 0:
                        continue
                    for hc in range(HC):
                        t = xtp.tile([128, JG], f32r, tag="xt")
                        nc.sync.dma_start(
                            t[:], xt[b, hc][:, jg * JG:(jg + 1) * JG])
                        xt_c[(hc, jg)] = t
                acc_prev = {}

                # phase 1: o_proj + tanh for BOTH jg units of this batch.
                # Interleaving the two units lets each unit's exp execute on
                # ScalarE while the other unit's scores matmuls run on PE, so
                # the ScalarE pipeline has no data-dependency stalls.
                en_sb = {}
                for jg in range(NJG):
                    for kc in range(HC):
                        ps = psp.tile([128, JG], f32, tag="ps")
                        for hc in range(HC):
                            for j4 in range(NJ4):
                                js = slice(j4 * J4, (j4 + 1) * J4)
                                if b == 0 and jg == 0:
                                    rhs = xt_first[(hc, j4)][:]
                                else:
                                    rhs = xt_c[(hc, jg)][:, js]
                                nc.tensor.matmul(
                                    ps[:, js],
                                    lhsT=w2t_sb[hc][:, kc * 128:(kc + 1) * 128],
                                    rhs=rhs,
                                    start=(hc == 0),
                                    stop=(hc == HC - 1),
                                )
                        sb = enp.tile([128, JG], fp16, tag="en")
                        nc.scalar.activation(
                            sb[:], ps[:], Tanh, bias=hproj_sb[kc][:, b:b + 1]
                        )
                        en_sb[(jg, kc)] = sb

                # phase 2: scores + exp + alpha + context for both units
                for jg in range(NJG):
                    jg0 = jg * JG
                    sc_ps = psp.tile([128, JG], f32, tag="ps")
                    for kc in range(HC):
                        for j4 in range(NJ4):
                            js = slice(j4 * J4, (j4 + 1) * J4)
                            nc.tensor.matmul(
                                sc_ps[:, js],
                                lhsT=vrep_sb[kc][:],
                                rhs=en_sb[(jg, kc)][:, js],
                                start=(kc == 0),
                                stop=(kc == HC - 1),
                            )
                    p_bc = pbp.tile([128, JG], f32, tag="pbc")
                    nc.scalar.activation(p_bc[:], sc_ps[:], Exp)

                    # unnormalized alpha row (gpsimd/SWDGE queue so it cannot
                    # head-of-line-block the xt prefetches on the sync queue)
                    nc.gpsimd.dma_start(alpha_u[b:b + 1, jg0:jg0 + JG],
                                        p_bc[0:1, :])

                    # context: ctx[h] += sum_s xT[h, s] * p[s]
                    # (custom DVE op: accum_out = s0 + sum(in0 * in1 * s1))
                    for hc in range(HC):
                        col = hc * BC + b
                        if b == 0 and jg == 0:
                            prev = None
                            for j4 in range(NJ4):
                                scr = scrp.tile([128, J4], f32, tag="scrf")
                                acc = accp.tile([128, 1], f32, tag="acc")
                                nc.vector._custom_dve(
                                    TENSOR_TENSOR_REDUCE,
                                    out=scr[:],
                                    in0=xt_first[(hc, j4)][:].bitcast(f32),
                                    in1=p_bc[:, j4 * J4:(j4 + 1) * J4],
                                    s0=(0.0 if prev is None else prev[:, 0:1]),
                                    s1=1.0,
                                    accum_out=acc[:, 0:1],
                                )
                                prev = acc
                            acc_prev[hc] = prev
                            continue
                        scr = scrp.tile([128, JG], f32, tag="scr")
                        if jg < NJG - 1:
                            acc = accp.tile([128, 1], f32, tag="acc")
                        else:
                            acc = None
                        nc.vector._custom_dve(
                            TENSOR_TENSOR_REDUCE,
                            out=scr[:],
                            in0=xt_c[(hc, jg)][:].bitcast(f32),
                            in1=p_bc[:],
                            s0=(0.0 if jg == 0 else acc_prev[hc][:, 0:1]),
                            s1=1.0,
                            accum_out=(ctx_sb[:, col:col + 1] if acc is None
                                       else acc[:, 0:1]),
                        )
                        if acc is not None:
                            acc_prev[hc] = acc

            nc.sync.dma_start(ctxu[:], ctx_sb[:])

    nc.compile()
    return nc


def _get_built():
    global _BUILT
    if _BUILT is None:
        _BUILT = _build()
    return _BUILT


def _install_ntff_hook():
    """Provide antenv.axon_hooks (absent on this image) so that
    run_bass_kernel_spmd(trace=True) can capture NTFF profiles."""
    import types
    import ctypes
    import contextlib

    if "antenv.axon_hooks" in sys.modules:
        return
    so_path = "/opt/axon/libaxon_pjrt.so"
    hook = None
    try:
        lib = ctypes.CDLL(so_path)
        if hasattr(lib, "axon_start_nrt_profile"):
            lib.axon_start_nrt_profile.argtypes = [
                ctypes.POINTER(ctypes.c_int64),
                ctypes.c_size_t,
            ]
            lib.axon_start_nrt_profile.restype = ctypes.c_int64
            lib.axon_stop_nrt_profile.argtypes = [ctypes.c_char_p]
            lib.axon_stop_nrt_profile.restype = ctypes.c_int64

            @contextlib.contextmanager
            def hook(output_dir, device_ids):
                import jax

                jax.devices()
                if device_ids:
                    ids = (ctypes.c_int64 * len(device_ids))(*device_ids)
                    rc = lib.axon_start_nrt_profile(ids, len(device_ids))
                else:
                    rc = lib.axon_start_nrt_profile(None, 0)
                if rc != 0:
                    raise RuntimeError(f"axon_start_nrt_profile rc={rc}")
                try:
                    yield
                finally:
                    n = lib.axon_stop_nrt_profile(str(output_dir).encode())
                    print(f"profile: {n} ntff file(s) in {output_dir}",
                          file=sys.stderr)
    except OSError:
        pass

    mod = types.ModuleType("antenv.axon_hooks")
    mod.get_axon_ntff_profile_hook = lambda: hook
    mod.set_axon_ntff_profile_hook = lambda h: None
    import antenv

    sys.modules["antenv.axon_hooks"] = mod
    antenv.axon_hooks = mod


def kernel(lstm_outputs, lstm_hidden_state, attn_w, attn_b, v_w):
    global LAST_EXEC_TIME_NS
    from concourse import bass_utils
    from concourse.bass_utils import run_bass_kernel_spmd

    trace = os.environ.get("KERNEL_TRACE", "0") == "1"
    if trace:
        _install_ntff_hook()
        bass_utils.upload_artifacts = lambda tmpdir: "local://" + tmpdir

    lstm_outputs = np.asarray(lstm_outputs, dtype=np.float32)
    hidden = np.asarray(lstm_hidden_state, dtype=np.float32)[-1]      # [B, H]
    attn_w = np.asarray(attn_w, dtype=np.float32)
    attn_b = np.asarray(attn_b, dtype=np.float32)
    v_w = np.asarray(v_w, dtype=np.float32)

    # host-side prep (sharding + layout). The fp32r-typed tensors carry full
    # fp32 bits: the PE RNE-rounds fp32r operands on read (verified on HW),
    # while the DVE context reduce reads the same tiles at full precision.
    xt_full = np.ascontiguousarray(lstm_outputs.transpose(0, 2, 1))   # [B, H, S]
    w2t_h = np.ascontiguousarray(attn_w[:, H:].T)                     # [H(h), H(k)]
    vrep_h = np.ascontiguousarray(
        np.broadcast_to(v_w[:, None], (H, 128))).astype(np.float16)
    hproj_full = hidden @ attn_w[:, :H].T + attn_b                    # [B, H]

    in_maps = []
    for c in range(NCORES):
        bs = slice(c * BC, (c + 1) * BC)
        in_maps.append({
            "xt": np.ascontiguousarray(
                xt_full[bs].reshape(BC, HC, 128, S)),
            "w2t": w2t_h,
            "vrep": vrep_h,
            "hproj": np.ascontiguousarray(hproj_full[bs].T),
        })

    nc = _get_built()
    res = run_bass_kernel_spmd(
        nc, in_maps, core_ids=list(range(NCORES)), trace=trace
    )
    LAST_EXEC_TIME_NS = res.exec_time_ns

    context = np.empty((B, H), dtype=np.float32)
    alpha = np.empty((B, S), dtype=np.float32)
    for c in range(NCORES):
        r = res.results[c]
        pu = r["alpha_u"]                                  # [BC, S]
        denom = pu.sum(axis=1, dtype=np.float32)           # [BC]
        bs = slice(c * BC, (c + 1) * BC)
        alpha[bs] = pu / denom[:, None]
        # ctxu[hp, hc*BC + b] = ctx_u[b, hc*128 + hp]
        cu = r["ctxu"].reshape(128, HC, BC)
        context[bs] = cu.transpose(2, 1, 0).reshape(BC, H) / denom[:, None]
    return (context, alpha)


# revision 17
# speedup vs baseline: 1.0100x; 1.0100x over previous
"""Bahdanau-style attention kernel for Trainium2, data-parallel over 8 NeuronCores.

Reference computation (per batch b):
    hidden  = lstm_hidden_state[-1]                      # [B, H]
    h_proj  = hidden @ attn_w[:, :H].T + attn_b          # [B, H]
    o_proj  = lstm_outputs @ attn_w[:, H:].T             # [B, S, H]
    energy  = tanh(o_proj + h_proj[:, None, :])          # [B, S, H]
    scores  = energy @ v_w                               # [B, S]
    alpha   = softmax(scores, axis=1)                    # [B, S]
    context = einsum('bs,bsh->bh', alpha, lstm_outputs)  # [B, H]
    returns (context, alpha)

Strategy:
  - Shard batch B=64 across 8 cores (8 batches each); weights replicated.
  - Host pre-transposes lstm_outputs to [B, H, S] so the device streams
    contiguous [128, S] tiles once (single pass over the 33.5MB/core shard).
  - o_projT[k, s] via f32r matmuls (W2T chunks stationary). The fp32r-typed
    tensors carry full fp32 bits: the PE RNE-rounds fp32r operands on read,
    so only the matmul path sees tf32-ish precision.
  - tanh(+h_proj bias) on ScalarE, PSUM -> SBUF, 2048-wide ops, fp16 output
    (tanh is bounded in [-1,1], where fp16 has 10-bit-mantissa accuracy).
  - scores via a replicated-v fp16 stationary operand so the score row comes
    out broadcast across all 128 partitions (needed by the context reduce).
  - p = exp(scores) on ScalarE (scores are bounded by ||v||_1 <= 16, so no
    max-subtraction is needed; softmax normalization happens on host).
  - context = sum_s xT[h, s] * p[s] as a fused custom-DVE reduce reading the
    full-precision x bits; the two jg-units of each batch are software-
    pipelined so ScalarE (the bottleneck engine) runs nearly gap-free.
"""

import os
import sys
import numpy as np

sys.path.insert(0, "/opt/trn_rl_repo")

B, S, H = 64, 4096, 256
NCORES = 8
BC = B // NCORES          # batches per core
HC = H // 128             # 128-partition h/k chunks per core
JG = 2048                 # free-dim width of ACT/DVE ops
NJG = S // JG
J4 = 512                  # matmul moving-operand width (fp32 max)
NJ4 = JG // J4

_BUILT = None
LAST_EXEC_TIME_NS = None


def _round_fp32r(x):
    """Round fp32 to the PE's fp32r (tf32-like) format: round-to-nearest-even
    at mantissa bit 12, low 12 bits zeroed."""
    u = np.ascontiguousarray(x, dtype=np.float32).view(np.uint32)
    keep = u & np.uint32(0xFFFFF000)
    rem = u & np.uint32(0x00000FFF)
    half = np.uint32(0x800)
    lsb = (u >> np.uint32(12)) & np.uint32(1)
    round_up = (rem > half) | ((rem == half) & (lsb == 1))
    out = keep + (round_up.astype(np.uint32) << np.uint32(12))
    return out.view(np.float32)


def _build():
    import concourse.tile as tile
    from concourse import bacc, mybir

    f32 = mybir.dt.float32
    f32r = mybir.dt.float32r
    fp16 = mybir.dt.float16
    Tanh = mybir.ActivationFunctionType.Tanh
    Exp = mybir.ActivationFunctionType.Exp
    mult = mybir.AluOpType.mult
    add = mybir.AluOpType.add

    nc = bacc.Bacc(
        "TRN2", target_bir_lowering=False, debug=False, num_devices=NCORES
    )

    xt = nc.dram_tensor("xt", [BC, HC, 128, S], f32r, kind="ExternalInput").ap()
    w2t = nc.dram_tensor("w2t", [H, H], f32r, kind="ExternalInput").ap()
    vrep = nc.dram_tensor("vrep", [H, 128], fp16, kind="ExternalInput").ap()
    hproj = nc.dram_tensor("hproj", [H, BC], f32, kind="ExternalInput").ap()
    alpha_u = nc.dram_tensor("alpha_u", [BC, S], f32, kind="ExternalOutput").ap()
    ctxu = nc.dram_tensor("ctxu", [128, HC * BC], f32, kind="ExternalOutput").ap()

    with tile.TileContext(nc) as tc:
        with (
            tc.tile_pool(name="singles", bufs=1) as singles,
            tc.tile_pool(name="xtp", bufs=12) as xtp,
            tc.tile_pool(name="enp", bufs=6) as enp,
            tc.tile_pool(name="pbp", bufs=4) as pbp,
            tc.tile_pool(name="scrp", bufs=2) as scrp,
            tc.tile_pool(name="accp", bufs=8) as accp,
            tc.tile_pool(name="psp", bufs=2, space="PSUM") as psp,
        ):
            # weights on the scalar HWDGE queue (issues in parallel with sync)
            w2t_sb = []
            vrep_sb = []
            hproj_sb = []
            for hc in range(HC):
                t = singles.tile([128, H], f32r, tag=f"w2t{hc}")
                nc.scalar.dma_start(t[:], w2t[hc * 128:(hc + 1) * 128, :])
                w2t_sb.append(t)
            for hc in range(HC):
                t = singles.tile([128, 128], fp16, tag=f"vrep{hc}")
                nc.scalar.dma_start(t[:], vrep[hc * 128:(hc + 1) * 128, :])
                vrep_sb.append(t)
                t = singles.tile([128, BC], f32, tag=f"hproj{hc}")
                nc.scalar.dma_start(t[:], hproj[hc * 128:(hc + 1) * 128, :])
                hproj_sb.append(t)
            ctx_sb = singles.tile([128, HC * BC], f32, tag="ctx")

            from concourse.dve_ops import TENSOR_TENSOR_REDUCE

            xt_first = {}  # (hc, j4) -> [128, J4] tile, for (b=0, jg=0)
            for j4 in range(NJ4):
                for hc in range(HC):
                    t = xtp.tile([128, J4], f32r, tag="xtf")
                    nc.sync.dma_start(
                        t[:], xt[0, hc][:, j4 * J4:(j4 + 1) * J4])
                    xt_first[(hc, j4)] = t

            for b in range(BC):
                xt_c = {}
                for jg in range(NJG):
                    if b == 0 and jg == 0:
                        continue
                    for hc in range(HC):
                        t = xtp.tile([128, JG], f32r, tag="xt")
                        nc.sync.dma_start(
                            t[:], xt[b, hc][:, jg * JG:(jg + 1) * JG])
                        xt_c[(hc, jg)] = t
                acc_prev = {}

                # phase 1: o_proj + tanh for BOTH jg units of this batch.
                # Interleaving the two units lets each unit's exp execute on
                # ScalarE while the other unit's scores matmuls run on PE, so
                # the ScalarE pipeline has no data-dependency stalls.
                en_sb = {}
                for jg in range(NJG):
                    for kc in range(HC):
                        ps = psp.tile([128, JG], f32, tag="ps")
                        for hc in range(HC):
                            for j4 in range(NJ4):
                                js = slice(j4 * J4, (j4 + 1) * J4)
                                if b == 0 and jg == 0:
                                    rhs = xt_first[(hc, j4)][:]
                                else:
                                    rhs = xt_c[(hc, jg)][:, js]
                                nc.tensor.matmul(
                                    ps[:, js],
                                    lhsT=w2t_sb[hc][:, kc * 128:(kc + 1) * 128],
                                    rhs=rhs,
                                    start=(hc == 0),
                                    stop=(hc == HC - 1),
                                )
                        sb = enp.tile([128, JG], fp16, tag="en")
                        nc.scalar.activation(
                            sb[:], ps[:], Tanh, bias=hproj_sb[kc][:, b:b + 1]
                        )
                        en_sb[(jg, kc)] = sb

                # phase 2: scores + exp + alpha + context for both units
                for jg in range(NJG):
                    jg0 = jg * JG
                    sc_ps = psp.tile([128, JG], f32, tag="ps")
                    for kc in range(HC):
                        for j4 in range(NJ4):
                            js = slice(j4 * J4, (j4 + 1) * J4)
                            nc.tensor.matmul(
                                sc_ps[:, js],
                                lhsT=vrep_sb[kc][:],
                                rhs=en_sb[(jg, kc)][:, js],
                                start=(kc == 0),
                                stop=(kc == HC - 1),
                            )
                    p_bc = pbp.tile([128, JG], f32, tag="pbc")
                    nc.scalar.activation(p_bc[:], sc_ps[:], Exp)

                    # unnormalized alpha row (gpsimd/SWDGE queue so it cannot
                    # head-of-line-block the xt prefetches on the sync queue)
                    nc.gpsimd.dma_start(alpha_u[b:b + 1, jg0:jg0 + JG],
                                        p_bc[0:1, :])

                    # context: ctx[h] += sum_s xT[h, s] * p[s]
                    # (custom DVE op: accum_out = s0 + sum(in0 * in1 * s1))
                    for hc in range(HC):
                        col = hc * BC + b
                        if b == 0 and jg == 0:
                            prev = None
                            for j4 in range(NJ4):
                                scr = scrp.tile([128, J4], f32, tag="scrf")
                                acc = accp.tile([128, 1], f32, tag="acc")
                                nc.vector._custom_dve(
                                    TENSOR_TENSOR_REDUCE,
                                    out=scr[:],
                                    in0=xt_first[(hc, j4)][:].bitcast(f32),
                                    in1=p_bc[:, j4 * J4:(j4 + 1) * J4],
                                    s0=(0.0 if prev is None else prev[:, 0:1]),
                                    s1=1.0,
                                    accum_out=acc[:, 0:1],
                                )
                                prev = acc
                            acc_prev[hc] = prev
                            continue
                        scr = scrp.tile([128, JG], f32, tag="scr")
                        if jg < NJG - 1:
                            acc = accp.tile([128, 1], f32, tag="acc")
                        else:
                            acc = None
                        nc.vector._custom_dve(
                            TENSOR_TENSOR_REDUCE,
                            out=scr[:],
                            in0=xt_c[(hc, jg)][:].bitcast(f32),
                            in1=p_bc[:],
                            s0=(0.0 if jg == 0 else acc_prev[hc][:, 0:1]),
                            s1=1.0,
                            accum_out=(ctx_sb[:, col:col + 1] if acc is None
                                       else acc[:, 0:1]),
                        )
                        if acc is not None:
                            acc_prev[hc] = acc

            nc.sync.dma_start(ctxu[:], ctx_sb[:])

    nc.compile()
    return nc


def _get_built():
    global _BUILT
    if _BUILT is None:
        _BUILT = _build()
    return _BUILT


def _install_ntff_hook():
    """Provide antenv.axon_hooks (absent on this image) so that
    run_bass_kernel_spmd(trace=True) can capture NTFF profiles."""
    import types
    import ctypes
    import contextlib

    if "antenv.axon_hooks" in sys.modules:
        return
    so_path = "/opt/axon/libaxon_pjrt.so"
    hook = None
    try:
        lib = ctypes.CDLL(so_path)
        if hasattr(lib, "axon_start_nrt_profile"):
            lib.axon_start_nrt_profile.argtypes = [
                ctypes.POINTER(ctypes.c_int64),
                ctypes.c_size_t,
            ]
            lib.axon_start_nrt_profile.restype = ctypes.c_int64
            lib.axon_stop_nrt_profile.argtypes = [ctypes.c_char_p]
            lib.axon_stop_nrt_profile.restype = ctypes.c_int64

            @contextlib.contextmanager
            def hook(output_dir, device_ids):
                import jax

                jax.devices()
                if device_ids:
                    ids = (ctypes.c_int64 * len(device_ids))(*device_ids)
                    rc = lib.axon_start_nrt_profile(ids, len(device_ids))
                else:
                    rc = lib.axon_start_nrt_profile(None, 0)
                if rc != 0:
                    raise RuntimeError(f"axon_start_nrt_profile rc={rc}")
                try:
                    yield
                finally:
                    n = lib.axon_stop_nrt_profile(str(output_dir).encode())
                    print(f"profile: {n} ntff file(s) in {output_dir}",
                          file=sys.stderr)
    except OSError:
        pass

    mod = types.ModuleType("antenv.axon_hooks")
    mod.get_axon_ntff_profile_hook = lambda: hook
    mod.set_axon_ntff_profile_hook = lambda h: None
    import antenv

    sys.modules["antenv.axon_hooks"] = mod
    antenv.axon_hooks = mod


def kernel(lstm_outputs, lstm_hidden_state, attn_w, attn_b, v_w):
    global LAST_EXEC_TIME_NS
    from concourse import bass_utils
    from concourse.bass_utils import run_bass_kernel_spmd

    trace = os.environ.get("KERNEL_TRACE", "0") == "1"
    if trace:
        _install_ntff_hook()
        bass_utils.upload_artifacts = lambda tmpdir: "local://" + tmpdir

    lstm_outputs = np.asarray(lstm_outputs, dtype=np.float32)
    hidden = np.asarray(lstm_hidden_state, dtype=np.float32)[-1]      # [B, H]
    attn_w = np.asarray(attn_w, dtype=np.float32)
    attn_b = np.asarray(attn_b, dtype=np.float32)
    v_w = np.asarray(v_w, dtype=np.float32)

    # host-side prep (sharding + layout). The fp32r-typed tensors carry full
    # fp32 bits: the PE RNE-rounds fp32r operands on read (verified on HW),
    # while the DVE context reduce reads the same tiles at full precision.
    xt_full = np.ascontiguousarray(lstm_outputs.transpose(0, 2, 1))   # [B, H, S]
    w2t_h = np.ascontiguousarray(attn_w[:, H:].T)                     # [H(h), H(k)]
    vrep_h = np.ascontiguousarray(
        np.broadcast_to(v_w[:, None], (H, 128))).astype(np.float16)
    hproj_full = hidden @ attn_w[:, :H].T + attn_b                    # [B, H]

    in_maps = []
    for c in range(NCORES):
        bs = slice(c * BC, (c + 1) * BC)
        in_maps.append({
            "xt": np.ascontiguousarray(
                xt_full[bs].reshape(BC, HC, 128, S)),
            "w2t": w2t_h,
            "vrep": vrep_h,
            "hproj": np.ascontiguousarray(hproj_full[bs].T),
        })

    nc = _get_built()
    res = run_bass_kernel_spmd(
        nc, in_maps, core_ids=list(range(NCORES)), trace=trace
    )
    LAST_EXEC_TIME_NS = res.exec_time_ns

    context = np.empty((B, H), dtype=np.float32)
    alpha = np.empty((B, S), dtype=np.float32)
    for c in range(NCORES):
        r = res.results[c]
        pu = r["alpha_u"]                                  # [BC, S]
        denom = pu.sum(axis=1, dtype=np.float32)           # [BC]
        bs = slice(c * BC, (c + 1) * BC)
        alpha[bs] = pu / denom[:, None]
        # ctxu[hp, hc*BC + b] = ctx_u[b, hc*128 + hp]
        cu = r["ctxu"].reshape(128, HC, BC)
        context[bs] = cu.transpose(2, 1, 0).reshape(BC, H) / denom[:, None]
    return (context, alpha)
